# revision 11
# baseline (speedup 1.0000x reference)
"""Trainium2 Bass kernel for nn_Graphs (soft decision-graph probability propagation).

Reference math (G=4 graphs, B=128 batch, N=255 internal nodes, L=256 leaves,
F=512 features, J=8 jumps):
  b  = sigmoid(x @ W_g^T + bias_g)                  (per graph: B x N)
  M0 = softmax(M_left, axis=dest), M1 = softmax(M_right, axis=dest)
  q  = [b*(M1-M0)+M0 | leaf-identity]               (per (g,batch): 511x511)
  prob <- q @ prob, J times, starting from e0; return leaf probs.

Restructure (v4c):
  - Softmax on the HOST (batch-independent weight preprocessing): device gets
    M0T = softmax(M_left)^T and DT = (softmax - softmax)^T; no exp / row-sum /
    reciprocal on device.
  - The internal-dest blocks are fp8 e4m3 scaled by 256 (sim rel-err 1.2e-2
    vs 2e-2 gate); the 1/256 rides in the cc coefficients so the state stays
    true-scale, and the leaf output is rescaled in the final cast.  The leaf
    blocks stay bf16 (fp8 there fails the gate: tiny leaf probs hit the
    quantization error directly).
  - One jump: u' = M0T' u + DT' v, v = b (.) u.  Per src tile t one DVE op
    makes both halves: uv[t] = cc[t] * broadcast(pq[t]), cc[t][:,0,:]=1/256,
    cc[t][:,1,:]=b/256 (f32: a bf16 cc measured 343ns vs 288 for the TT).
    8 K=128 N=64 matmuls per jump; pq'[0]'s stop ordered early.
  - Leaf block hoisted: leaf = [M0TL|DTL]^T Sacc, Sacc = sum_j uv_j (bf16,
    sim err 8.5e-3): sacc[0] accumulated by gpsimd, sacc[1] by DVE --
    one engine doing both (2x ~512ns) lagged the loop by ~1us.
  - b = sigmoid(logit + bias) on ACT.  A dummy activation right after the
    memsets hoists the auto-inserted 1.5us ACT table load to program start
    (v4b: the load ran lazily right before the first sigmoid, serializing
    the front by ~1.5us).
  - DMA: wx is split across BOTH queues (sync: k01, gpsimd: k23) so the
    b-matmul gate lands ~1us earlier; eint (fp8, 128KB) follows on gpsimd,
    bias + eleaf on sync.
  - PE warm-up: 6 N=512 matmuls (~3.1us cold).  v4b's 8 ended ~1.3us after
    the wx sem and pushed the b-matmuls out ~2us (PE queue is FIFO).

Sharding: 8 cores = (graph g = core//2) x (batch half h = core%2, 64 rows).
No cross-core communication. Host layouts:
  - mint (128,1024) fp8e4m3 = 256*[M0T-int | DT-int] per src tile (255 real
    dest + zero pad col; src pad row zero).
  - mleaf (128,1024) bf16: [M0TL | DTL] per src tile.
  - wxp (128,1280) bf16: per F-tile k, cols [320k:320k+256] = W_g^T block,
    [320k+256:320k+320] = x_half^T block.
  - biasp (128,2) f32: bias node-tiled.
Output per core: (64,256) bf16 leaf-major; host assembles to (B,L,G) and
applies the reference interval clamp.
"""

import numpy as np
import ml_dtypes

G, B, N, L, F, J = 4, 128, 255, 256, 512, 8
BH = B // 2  # 64 batch rows per core
NCORES = 8
BF16 = ml_dtypes.bfloat16
E4M3 = ml_dtypes.float8_e4m3
MS = 256.0  # fp8 scale for the internal-dest matrices

_CACHE = {}


def _build_program():
    import concourse.mybir as mybir
    from concourse import bacc
    from concourse.tile import TileContext

    f32 = mybir.dt.float32
    bf16 = mybir.dt.bfloat16
    f8 = mybir.dt.float8e4
    AF = mybir.ActivationFunctionType
    mult = mybir.AluOpType.mult

    nc = bacc.Bacc(None)
    p_mint = nc.declare_dram_parameter("mint", [128, 1024], f8, isOutput=False)
    p_mleaf = nc.declare_dram_parameter("mleaf", [128, 1024], bf16, isOutput=False)
    p_wx = nc.declare_dram_parameter("wxp", [128, 1280], bf16, isOutput=False)
    p_bias = nc.declare_dram_parameter("biasp", [128, 2], f32, isOutput=False)
    p_out = nc.declare_dram_parameter("out", [BH, 256], bf16, isOutput=True)

    with TileContext(nc) as tc:
        with (
            tc.tile_pool(name="consts", bufs=1) as consts,
            tc.tile_pool(name="work", bufs=2) as work,
            tc.tile_pool(name="state", bufs=4) as state,
            tc.tile_pool(name="psum", bufs=2, space="PSUM") as psum,
            tc.tile_pool(name="psum_acc", bufs=1, space="PSUM") as psum_acc,
            tc.tile_pool(name="psum_w", bufs=1, space="PSUM") as psum_w,
        ):
            # ---- DMA issue (first: these gate everything) ----
            eint = consts.tile([128, 1024], f8, tag="eint", name="eint")
            wx = consts.tile([128, 1280], bf16, tag="wx", name="wx")
            bias = consts.tile([128, 2], f32, tag="bias", name="bias")
            eleaf = consts.tile([128, 1024], bf16, tag="eleaf", name="eleaf")
            nc.sync.dma_start(wx[:, 0:640], p_wx[:, 0:640])
            nc.sync.dma_start(bias[:], p_bias[:, :])
            nc.sync.dma_start(eleaf[:], p_mleaf[:, :])
            nc.gpsimd.dma_start(wx[:, 640:1280], p_wx[:, 640:1280])
            nc.gpsimd.dma_start(eint[:], p_mint[:, :])

            # ---- PE warm-up (HAM un-throttle) ----
            wsc = consts.tile([128, 128], bf16, tag="wsc", name="wsc")
            rsc = consts.tile([128, 512], bf16, tag="rsc", name="rsc")
            nc.vector.memset(wsc[:], 0.0)
            nc.vector.memset(rsc[:], 0.0)
            # dummy activation: hoists the auto-inserted ACT table load to
            # program start (it precedes this inst on the Scalar queue and
            # has no waits)
            dum = work.tile([1, 1], f32, tag="dum", name="dum")
            nc.scalar.activation(dum[:], wsc[0:1, 0:1], AF.Sigmoid)
            pw = psum_w.tile([128, 512], f32, tag="pw", name="pw")
            pb = psum.tile([128, 2, BH], f32, tag="pb", name="pb")
            pleaf = psum_acc.tile([BH, 256], f32, tag="pl", name="pl")
            for _ in range(5):
                nc.tensor.matmul(pw[:], wsc[:], rsc[:], start=True, stop=True)
            zw = work.tile([128, 1], f32, tag="zw", name="zw")
            nc.vector.tensor_scalar_mul(zw[:], pw[:, 0:1], 0.0)
            nc.vector.tensor_scalar_mul(rsc[0:1, 0:1], zw[0:1, :], 0.0)
            pq = [psum.tile([128, BH], f32, tag=f"pq{mt}", name=f"pq{mt}") for mt in range(2)]
            for mt in range(2):
                nc.tensor.matmul(pq[mt][:], wsc[:], rsc[:, 0:BH], start=True, stop=True)

            # ---- b = sigmoid(W @ x^T + bias) -> cc[t][:,1,:] = b/256 ----
            for mh in range(2):
                for k in range(4):
                    nc.tensor.matmul(
                        pb[:, mh, :],
                        wx[:, k * 320 + mh * 128:k * 320 + (mh + 1) * 128],
                        wx[:, k * 320 + 256:k * 320 + 320],
                        start=(k == 0), stop=(k == 3),
                    )
            cc = [consts.tile([128, 2, BH], f32, tag=f"cc{t}", name=f"cc{t}") for t in range(2)]
            ccr0 = state.tile([1, 2, BH], bf16, tag="ccr0", name="ccr0")
            nc.vector.memset(ccr0[0:1, 0, :], 1.0 / MS)
            for t in range(2):
                nc.vector.memset(cc[t][:, 0, :], 1.0 / MS)
                nc.scalar.activation(cc[t][:, 1, :], pb[:, t, :], AF.Sigmoid,
                                     bias=bias[:, t:t + 1])
                if t == 0:
                    # jump-0 rhs: b[node0]/256 in bf16, straight off the
                    # sigmoid output (reads the pre-scale value; WAR keeps
                    # it ordered before the in-place scale below)
                    nc.vector.tensor_scalar_mul(ccr0[0:1, 1, :],
                                                cc[0][0:1, 1, :], 1.0 / MS)
                nc.vector.tensor_scalar_mul(cc[t][:, 1, :], cc[t][:, 1, :], 1.0 / MS)

            # ---- leaf-sum accumulators (bf16; uv is already /256) ----
            # Seed = jump 0's uv row: u_0 = e0 -> sacc[0][0,:,:] = (1,b)/256.
            sacc = [consts.tile([128, 2, BH], bf16, tag=f"sacc{t}", name=f"sacc{t}") for t in range(2)]
            nc.gpsimd.memset(sacc[0][:], 0.0)
            nc.vector.memset(sacc[1][:], 0.0)
            nc.gpsimd.tensor_add(sacc[0][0:1, :, :], sacc[0][0:1, :, :], cc[0][0:1, :, :])

            # ---- jump 0: u_1 = M0T[0,:] + DT[0,:]*b[0,:] (true scale) ----
            for mt in range(2):
                nc.tensor.matmul(pq[mt][:], eint[0:1, mt * 128:(mt + 1) * 128],
                                 ccr0[0:1, 0, :], start=True, stop=False)
                nc.tensor.matmul(pq[mt][:], eint[0:1, 256 + mt * 128:256 + (mt + 1) * 128],
                                 ccr0[0:1, 1, :], start=False, stop=True)

            # ---- jump loop ----
            for j in range(1, J):
                uv = [state.tile([128, 2, BH], bf16, tag=f"uv{t}", name=f"uv{t}") for t in range(2)]
                last = j == J - 1
                for t in range(2):
                    nc.vector.tensor_tensor(
                        out=uv[t][:], in0=cc[t][:],
                        in1=pq[t][:, None, :].broadcast_to([128, 2, BH]), op=mult)
                # sacc[0] on gpsimd, sacc[1] on DVE (measured best: both
                # on gpsimd paces the late jumps to ~1055ns)
                nc.gpsimd.tensor_add(sacc[0][:], sacc[0][:], uv[0][:])
                nc.vector.tensor_add(sacc[1][:], sacc[1][:], uv[1][:])
                if last:
                    break
                pq = [psum.tile([128, BH], f32, tag=f"pq{mt}", name=f"pq{mt}") for mt in range(2)]
                nc.tensor.matmul(pq[0][:], eint[:, 0:128], uv[0][:, 0, :],
                                 start=True, stop=False)
                nc.tensor.matmul(pq[0][:], eint[:, 256:384], uv[0][:, 1, :],
                                 start=False, stop=False)
                nc.tensor.matmul(pq[1][:], eint[:, 128:256], uv[0][:, 0, :],
                                 start=True, stop=False)
                nc.tensor.matmul(pq[1][:], eint[:, 384:512], uv[0][:, 1, :],
                                 start=False, stop=False)
                nc.tensor.matmul(pq[0][:], eint[:, 512:640], uv[1][:, 0, :],
                                 start=False, stop=False)
                nc.tensor.matmul(pq[0][:], eint[:, 768:896], uv[1][:, 1, :],
                                 start=False, stop=True)
                nc.tensor.matmul(pq[1][:], eint[:, 640:768], uv[1][:, 0, :],
                                 start=False, stop=False)
                nc.tensor.matmul(pq[1][:], eint[:, 896:1024], uv[1][:, 1, :],
                                 start=False, stop=True)

            # keep the PE busy across the j7 elementwise window (the leaf
            # matmuls otherwise pace at the cold 213ns N=256 rate)
            for _ in range(2):
                nc.tensor.matmul(pleaf[:], wsc[:, 0:BH], rsc[:, 0:256],
                                 start=True, stop=True)

            # ---- leaf block (once; output is /256, rescaled in the cast) ----
            nc.tensor.matmul(pleaf[:], sacc[0][:, 0, :], eleaf[:, 0:256],
                             start=True, stop=False)
            nc.tensor.matmul(pleaf[:], sacc[0][:, 1, :], eleaf[:, 256:512],
                             start=False, stop=False)
            nc.tensor.matmul(pleaf[:], sacc[1][:, 0, :], eleaf[:, 512:768],
                             start=False, stop=False)
            nc.tensor.matmul(pleaf[:], sacc[1][:, 1, :], eleaf[:, 768:1024],
                             start=False, stop=True)

            # ---- output (cast split across DVE + ACT, x256 rescale) ----
            o = work.tile([BH, 256], bf16, tag="o", name="o")
            nc.vector.tensor_scalar_mul(o[:], pleaf[:], MS)
            nc.sync.dma_start(p_out[:, :], o[:])

    nc.finalize()
    return nc


def _get_program():
    if "nc" not in _CACHE:
        _CACHE["nc"] = _build_program()
    return _CACHE["nc"]


def _softmax0(m):
    e = np.exp(m - m.max(axis=0, keepdims=True))
    return e / e.sum(axis=0, keepdims=True)


def _prep_inputs(x, W, bias, M_left, M_right):
    """Host-side shard + layout prep. Core c -> graph c//2, batch half c%2."""
    in_maps = []
    mint_g, mleaf_g, wt_g, bias_g = [], [], [], []
    for g in range(G):
        M0 = _softmax0(M_left[g].astype(np.float64))   # (511, 255)
        M1 = _softmax0(M_right[g].astype(np.float64))
        M0T = M0.T.astype(np.float32)                  # (255, 511) src-major
        DT = (M1 - M0).T.astype(np.float32)
        mint = np.zeros((128, 1024), np.float32)
        mleaf = np.zeros((128, 1024), np.float32)
        for t in range(2):
            rows = slice(t * 128, min((t + 1) * 128, 255))
            nr = rows.stop - rows.start
            base = t * 512
            mint[0:nr, base:base + 255] = M0T[rows, 0:255]
            mint[0:nr, base + 256:base + 511] = DT[rows, 0:255]
            mleaf[0:nr, base:base + 256] = M0T[rows, 255:511]
            mleaf[0:nr, base + 256:base + 512] = DT[rows, 255:511]
        mint_g.append((mint * np.float32(MS)).astype(E4M3))
        mleaf_g.append(mleaf.astype(BF16))
        wt = np.zeros((512, 256), np.float32)
        wt[:, 0:255] = W[g].T
        wt_g.append(wt)
        bp = np.zeros((256,), np.float32)
        bp[0:255] = bias[g]
        bias_g.append(np.ascontiguousarray(bp.reshape(2, 128).T))  # (128, 2)
    xt_h = [np.ascontiguousarray(x[h * BH:(h + 1) * BH].T) for h in range(2)]
    for c in range(NCORES):
        g, h = c // 2, c % 2
        wxc = np.concatenate([wt_g[g], xt_h[h]], axis=1)  # (512, 320)
        wxp = np.ascontiguousarray(
            wxc.reshape(4, 128, 320).transpose(1, 0, 2).reshape(128, 1280)
        ).astype(BF16)
        in_maps.append({
            "mint": mint_g[g], "mleaf": mleaf_g[g], "wxp": wxp,
            "biasp": bias_g[g],
        })
    return in_maps


def _assemble(results):
    eps = np.float32(1e-5)
    ret = np.empty((B, L, G), np.float32)
    for c in range(NCORES):
        g, h = c // 2, c % 2
        ret[h * BH:(h + 1) * BH, :, g] = results[c]["out"].astype(np.float32)
    ret = np.where(ret > 0.0, ret, eps)
    ret = np.where(ret < 1.0, ret, np.float32(1.0) - eps)
    return ret.astype(np.float32)


def run_on_device(in_maps, trace=False, **kw):
    from concourse.bass_utils import run_bass_kernel_spmd
    nc = _get_program()
    return run_bass_kernel_spmd(nc, in_maps, list(range(NCORES)), trace=trace, **kw)


def kernel(x, W, bias, M_left, M_right):
    in_maps = _prep_inputs(
        np.asarray(x, np.float32), np.asarray(W, np.float32),
        np.asarray(bias, np.float32), np.asarray(M_left, np.float32),
        np.asarray(M_right, np.float32),
    )
    res = run_on_device(in_maps)
    return _assemble(res.results)


# revision 12
# speedup vs baseline: 1.0322x; 1.0322x over previous
"""Trainium2 Bass kernel for nn_Graphs (soft decision-graph probability propagation).

Reference math (G=4 graphs, B=128 batch, N=255 internal nodes, L=256 leaves,
F=512 features, J=8 jumps):
  b  = sigmoid(x @ W_g^T + bias_g)                  (per graph: B x N)
  M0 = softmax(M_left, axis=dest), M1 = softmax(M_right, axis=dest)
  q  = [b*(M1-M0)+M0 | leaf-identity]               (per (g,batch): 511x511)
  prob <- q @ prob, J times, starting from e0; return leaf probs.

Restructure (v4c):
  - Softmax on the HOST (batch-independent weight preprocessing): device gets
    M0T = softmax(M_left)^T and DT = (softmax - softmax)^T; no exp / row-sum /
    reciprocal on device.
  - The internal-dest blocks are fp8 e4m3 scaled by 256 (sim rel-err 1.2e-2
    vs 2e-2 gate); the 1/256 rides in the cc coefficients so the state stays
    true-scale, and the leaf output is rescaled in the final cast.  The leaf
    blocks stay bf16 (fp8 there fails the gate: tiny leaf probs hit the
    quantization error directly).
  - One jump: u' = M0T' u + DT' v, v = b (.) u.  Per src tile t one DVE op
    makes both halves: uv[t] = cc[t] * broadcast(pq[t]), cc[t][:,0,:]=1/256,
    cc[t][:,1,:]=b/256 (f32: a bf16 cc measured 343ns vs 288 for the TT).
    8 K=128 N=64 matmuls per jump; pq'[0]'s stop ordered early.
  - Leaf block hoisted: leaf = [M0TL|DTL]^T Sacc, Sacc = sum_j uv_j (bf16,
    sim err 8.5e-3): sacc[0] accumulated by gpsimd, sacc[1] by DVE --
    one engine doing both (2x ~512ns) lagged the loop by ~1us.
  - b = sigmoid(logit + bias) on ACT.  A dummy activation right after the
    memsets hoists the auto-inserted 1.5us ACT table load to program start
    (v4b: the load ran lazily right before the first sigmoid, serializing
    the front by ~1.5us).
  - DMA: wx is split across BOTH queues (sync: k01, gpsimd: k23) so the
    b-matmul gate lands ~1us earlier; eint (fp8, 128KB) follows on gpsimd,
    bias + eleaf on sync.
  - PE warm-up: 6 N=512 matmuls (~3.1us cold).  v4b's 8 ended ~1.3us after
    the wx sem and pushed the b-matmuls out ~2us (PE queue is FIFO).

Sharding: 8 cores = (graph g = core//2) x (batch half h = core%2, 64 rows).
No cross-core communication. Host layouts:
  - mint (128,1024) fp8e4m3 = 256*[M0T-int | DT-int] per src tile (255 real
    dest + zero pad col; src pad row zero).
  - mleaf (128,1024) bf16: [M0TL | DTL] per src tile.
  - wxp (128,1280) bf16: per F-tile k, cols [320k:320k+256] = W_g^T block,
    [320k+256:320k+320] = x_half^T block.
  - biasp (128,2) f32: bias node-tiled.
Output per core: (64,256) bf16 leaf-major; host assembles to (B,L,G) and
applies the reference interval clamp.
"""

import numpy as np
import ml_dtypes

G, B, N, L, F, J = 4, 128, 255, 256, 512, 8
BH = B // 2  # 64 batch rows per core
NCORES = 8
BF16 = ml_dtypes.bfloat16
E4M3 = ml_dtypes.float8_e4m3
MS = 256.0  # fp8 scale for the internal-dest matrices

_CACHE = {}


def _build_program():
    import concourse.mybir as mybir
    from concourse import bacc
    from concourse.tile import TileContext

    f32 = mybir.dt.float32
    bf16 = mybir.dt.bfloat16
    f8 = mybir.dt.float8e4
    AF = mybir.ActivationFunctionType
    mult = mybir.AluOpType.mult

    nc = bacc.Bacc(None)
    p_mint = nc.declare_dram_parameter("mint", [128, 1024], f8, isOutput=False)
    p_mleaf = nc.declare_dram_parameter("mleaf", [128, 1024], bf16, isOutput=False)
    p_wx = nc.declare_dram_parameter("wxp", [128, 1280], bf16, isOutput=False)
    p_bias = nc.declare_dram_parameter("biasp", [128, 2], f32, isOutput=False)
    p_out = nc.declare_dram_parameter("out", [BH, 256], bf16, isOutput=True)

    with TileContext(nc) as tc:
        with (
            tc.tile_pool(name="consts", bufs=1) as consts,
            tc.tile_pool(name="work", bufs=2) as work,
            tc.tile_pool(name="state", bufs=4) as state,
            tc.tile_pool(name="psum", bufs=2, space="PSUM") as psum,
            tc.tile_pool(name="psum_acc", bufs=1, space="PSUM") as psum_acc,
            tc.tile_pool(name="psum_w", bufs=1, space="PSUM") as psum_w,
        ):
            # ---- DMA issue (first: these gate everything) ----
            eint = consts.tile([128, 1024], f8, tag="eint", name="eint")
            wx = consts.tile([128, 1280], bf16, tag="wx", name="wx")
            bias = consts.tile([128, 2], f32, tag="bias", name="bias")
            eleaf = consts.tile([128, 1024], bf16, tag="eleaf", name="eleaf")
            nc.sync.dma_start(wx[:, 0:640], p_wx[:, 0:640])
            nc.sync.dma_start(bias[:], p_bias[:, :])
            nc.sync.dma_start(eleaf[:], p_mleaf[:, :])
            nc.gpsimd.dma_start(wx[:, 640:1280], p_wx[:, 640:1280])
            nc.gpsimd.dma_start(eint[:], p_mint[:, :])

            # ---- PE warm-up (HAM un-throttle) ----
            wsc = consts.tile([128, 128], bf16, tag="wsc", name="wsc")
            rsc = consts.tile([128, 512], bf16, tag="rsc", name="rsc")
            nc.vector.memset(wsc[:], 0.0)
            nc.vector.memset(rsc[:], 0.0)
            # dummy activation: hoists the auto-inserted ACT table load to
            # program start (it precedes this inst on the Scalar queue and
            # has no waits)
            dum = work.tile([1, 1], f32, tag="dum", name="dum")
            nc.scalar.activation(dum[:], wsc[0:1, 0:1], AF.Sigmoid)
            pw = psum_w.tile([128, 512], f32, tag="pw", name="pw")
            pb = psum.tile([128, 2, BH], f32, tag="pb", name="pb")
            pleaf = psum_acc.tile([BH, 256], f32, tag="pl", name="pl")
            for _ in range(5):
                nc.tensor.matmul(pw[:], wsc[:], rsc[:], start=True, stop=True)
            zw = work.tile([128, 1], f32, tag="zw", name="zw")
            nc.vector.tensor_scalar_mul(zw[:], pw[:, 0:1], 0.0)
            nc.vector.tensor_scalar_mul(rsc[0:1, 0:1], zw[0:1, :], 0.0)
            for mh in range(2):
                nc.tensor.matmul(pb[:, mh, :], wsc[:], rsc[:, 0:BH],
                                 start=True, stop=True)
            pq = [psum.tile([128, BH], f32, tag=f"pq{mt}", name=f"pq{mt}") for mt in range(2)]
            for mt in range(2):
                nc.tensor.matmul(pq[mt][:], wsc[:], rsc[:, 0:BH], start=True, stop=True)

            # ---- b = sigmoid(W @ x^T + bias) -> cc[t][:,1,:] = b/256 ----
            for mh in range(2):
                for k in range(4):
                    nc.tensor.matmul(
                        pb[:, mh, :],
                        wx[:, k * 320 + mh * 128:k * 320 + (mh + 1) * 128],
                        wx[:, k * 320 + 256:k * 320 + 320],
                        start=(k == 0), stop=(k == 3),
                    )
            cc = [consts.tile([128, 2, BH], f32, tag=f"cc{t}", name=f"cc{t}") for t in range(2)]
            ccr0 = state.tile([1, 2, BH], bf16, tag="ccr0", name="ccr0")
            nc.vector.memset(ccr0[0:1, 0, :], 1.0 / MS)
            for t in range(2):
                nc.vector.memset(cc[t][:, 0, :], 1.0 / MS)
                nc.scalar.activation(cc[t][:, 1, :], pb[:, t, :], AF.Sigmoid,
                                     bias=bias[:, t:t + 1])
                if t == 0:
                    # jump-0 rhs: b[node0]/256 in bf16, straight off the
                    # sigmoid output (reads the pre-scale value; WAR keeps
                    # it ordered before the in-place scale below)
                    nc.vector.tensor_scalar_mul(ccr0[0:1, 1, :],
                                                cc[0][0:1, 1, :], 1.0 / MS)
                nc.vector.tensor_scalar_mul(cc[t][:, 1, :], cc[t][:, 1, :], 1.0 / MS)

            # ---- leaf-sum accumulators (bf16; uv is already /256) ----
            # Seed = jump 0's uv row: u_0 = e0 -> sacc[0][0,:,:] = (1,b)/256.
            sacc = [consts.tile([128, 2, BH], bf16, tag=f"sacc{t}", name=f"sacc{t}") for t in range(2)]
            nc.gpsimd.memset(sacc[0][:], 0.0)
            nc.vector.memset(sacc[1][:], 0.0)
            nc.gpsimd.tensor_add(sacc[0][0:1, :, :], sacc[0][0:1, :, :], cc[0][0:1, :, :])

            # ---- jump 0: u_1 = M0T[0,:] + DT[0,:]*b[0,:] (true scale) ----
            for mt in range(2):
                nc.tensor.matmul(pq[mt][:], eint[0:1, mt * 128:(mt + 1) * 128],
                                 ccr0[0:1, 0, :], start=True, stop=False)
                nc.tensor.matmul(pq[mt][:], eint[0:1, 256 + mt * 128:256 + (mt + 1) * 128],
                                 ccr0[0:1, 1, :], start=False, stop=True)

            # ---- jump loop ----
            for j in range(1, J):
                uv = [state.tile([128, 2, BH], bf16, tag=f"uv{t}", name=f"uv{t}") for t in range(2)]
                last = j == J - 1
                for t in range(2):
                    nc.vector.tensor_tensor(
                        out=uv[t][:], in0=cc[t][:],
                        in1=pq[t][:, None, :].broadcast_to([128, 2, BH]), op=mult)
                # sacc[0] on gpsimd, sacc[1] on DVE (measured best: both
                # on gpsimd paces the late jumps to ~1055ns)
                nc.gpsimd.tensor_add(sacc[0][:], sacc[0][:], uv[0][:])
                nc.vector.tensor_add(sacc[1][:], sacc[1][:], uv[1][:])
                if last:
                    break
                pq = [psum.tile([128, BH], f32, tag=f"pq{mt}", name=f"pq{mt}") for mt in range(2)]
                nc.tensor.matmul(pq[0][:], eint[:, 0:128], uv[0][:, 0, :],
                                 start=True, stop=False)
                nc.tensor.matmul(pq[0][:], eint[:, 256:384], uv[0][:, 1, :],
                                 start=False, stop=False)
                nc.tensor.matmul(pq[1][:], eint[:, 128:256], uv[0][:, 0, :],
                                 start=True, stop=False)
                nc.tensor.matmul(pq[1][:], eint[:, 384:512], uv[0][:, 1, :],
                                 start=False, stop=False)
                nc.tensor.matmul(pq[0][:], eint[:, 512:640], uv[1][:, 0, :],
                                 start=False, stop=False)
                nc.tensor.matmul(pq[0][:], eint[:, 768:896], uv[1][:, 1, :],
                                 start=False, stop=True)
                nc.tensor.matmul(pq[1][:], eint[:, 640:768], uv[1][:, 0, :],
                                 start=False, stop=False)
                nc.tensor.matmul(pq[1][:], eint[:, 896:1024], uv[1][:, 1, :],
                                 start=False, stop=True)

            # ---- leaf block (once; output is /256, rescaled in the cast) ----
            nc.tensor.matmul(pleaf[:], sacc[0][:, 0, :], eleaf[:, 0:256],
                             start=True, stop=False)
            nc.tensor.matmul(pleaf[:], sacc[0][:, 1, :], eleaf[:, 256:512],
                             start=False, stop=False)
            nc.tensor.matmul(pleaf[:], sacc[1][:, 0, :], eleaf[:, 512:768],
                             start=False, stop=False)
            nc.tensor.matmul(pleaf[:], sacc[1][:, 1, :], eleaf[:, 768:1024],
                             start=False, stop=True)

            # ---- output (cast split across DVE + ACT, x256 rescale) ----
            o = work.tile([BH, 256], bf16, tag="o", name="o")
            nc.vector.tensor_scalar_mul(o[:], pleaf[:], MS)
            nc.sync.dma_start(p_out[:, :], o[:])

    nc.finalize()
    return nc


def _get_program():
    if "nc" not in _CACHE:
        _CACHE["nc"] = _build_program()
    return _CACHE["nc"]


def _softmax0(m):
    e = np.exp(m - m.max(axis=0, keepdims=True))
    return e / e.sum(axis=0, keepdims=True)


def _prep_inputs(x, W, bias, M_left, M_right):
    """Host-side shard + layout prep. Core c -> graph c//2, batch half c%2."""
    in_maps = []
    mint_g, mleaf_g, wt_g, bias_g = [], [], [], []
    for g in range(G):
        M0 = _softmax0(M_left[g].astype(np.float64))   # (511, 255)
        M1 = _softmax0(M_right[g].astype(np.float64))
        M0T = M0.T.astype(np.float32)                  # (255, 511) src-major
        DT = (M1 - M0).T.astype(np.float32)
        mint = np.zeros((128, 1024), np.float32)
        mleaf = np.zeros((128, 1024), np.float32)
        for t in range(2):
            rows = slice(t * 128, min((t + 1) * 128, 255))
            nr = rows.stop - rows.start
            base = t * 512
            mint[0:nr, base:base + 255] = M0T[rows, 0:255]
            mint[0:nr, base + 256:base + 511] = DT[rows, 0:255]
            mleaf[0:nr, base:base + 256] = M0T[rows, 255:511]
            mleaf[0:nr, base + 256:base + 512] = DT[rows, 255:511]
        mint_g.append((mint * np.float32(MS)).astype(E4M3))
        mleaf_g.append(mleaf.astype(BF16))
        wt = np.zeros((512, 256), np.float32)
        wt[:, 0:255] = W[g].T
        wt_g.append(wt)
        bp = np.zeros((256,), np.float32)
        bp[0:255] = bias[g]
        bias_g.append(np.ascontiguousarray(bp.reshape(2, 128).T))  # (128, 2)
    xt_h = [np.ascontiguousarray(x[h * BH:(h + 1) * BH].T) for h in range(2)]
    for c in range(NCORES):
        g, h = c // 2, c % 2
        wxc = np.concatenate([wt_g[g], xt_h[h]], axis=1)  # (512, 320)
        wxp = np.ascontiguousarray(
            wxc.reshape(4, 128, 320).transpose(1, 0, 2).reshape(128, 1280)
        ).astype(BF16)
        in_maps.append({
            "mint": mint_g[g], "mleaf": mleaf_g[g], "wxp": wxp,
            "biasp": bias_g[g],
        })
    return in_maps


def _assemble(results):
    eps = np.float32(1e-5)
    ret = np.empty((B, L, G), np.float32)
    for c in range(NCORES):
        g, h = c // 2, c % 2
        ret[h * BH:(h + 1) * BH, :, g] = results[c]["out"].astype(np.float32)
    ret = np.where(ret > 0.0, ret, eps)
    ret = np.where(ret < 1.0, ret, np.float32(1.0) - eps)
    return ret.astype(np.float32)


def run_on_device(in_maps, trace=False, **kw):
    from concourse.bass_utils import run_bass_kernel_spmd
    nc = _get_program()
    return run_bass_kernel_spmd(nc, in_maps, list(range(NCORES)), trace=trace, **kw)


def kernel(x, W, bias, M_left, M_right):
    in_maps = _prep_inputs(
        np.asarray(x, np.float32), np.asarray(W, np.float32),
        np.asarray(bias, np.float32), np.asarray(M_left, np.float32),
        np.asarray(M_right, np.float32),
    )
    res = run_on_device(in_maps)
    return _assemble(res.results)
